# revision 47
# baseline (speedup 1.0000x reference)
"""Enformer relative-position attention on 8 Trainium2 NeuronCores.

Sharding: core c handles batch b = c//2 and head group g = c%2 (4 heads).
Each core computes its head-group's slice of the attention and a partial
output projection (contraction over its 768 value channels); the host sums
the two partials per batch. The bias b_out is added on even cores only.

relative_shift is implemented with a DRAM stride trick: the per-row-chunk
rel-logit band [128, 1664] is written to scratch DRAM with row stride 1664
and read back with row stride 1663 at offset 127, which realizes
out[p, j] = band[p, j - p + 127] exactly.

v2 changes vs. the 649us baseline:
 - all big tensors (x, weights, positional embed, output) live in DRAM as
   fp16: halves the HBM traffic of the load/store phases.
 - the band prepass is interleaved into the projection phase (after each
   window's qT columns are ready) so the tensor engine stream stays dense
   and the HAM clock-gate stays at 8/8.
 - the shifted band is added into the content-logit PSUM with identity
   matmuls instead of a vector ADD: drops ~85us of DVE time and takes the
   DVE out of the softmax critical path.
 - exp writes fp16 (so the softmax rescale runs in the DVE 4x mode) and
   PSUM evacuations are spread across vector/scalar/gpsimd.
"""
import math
import sys
from contextlib import ExitStack

import numpy as np

for _p in ("/opt/trn_rl_repo",):
    if _p not in sys.path:
        sys.path.append(_p)

import concourse.bass as bass
import concourse.mybir as mybir
import concourse.tile as tile
from concourse.bass_utils import run_bass_kernel_spmd
from concourse.masks import make_identity

F32 = mybir.dt.float32
F16 = mybir.dt.float16
BF16 = mybir.dt.bfloat16

B, N, D = 4, 1536, 1536
H, DK, DV, NRPF = 8, 64, 192, 192
HG = 4                  # heads per core
F = HG * DK             # 256 q/k columns per core
DVG = HG * DV           # 768 v columns per core
R = 2 * N - 1           # 3071 relative positions
RP = 3072               # padded
BW = 1664               # band window width per 128-row chunk (1663 + 1 pad)
NT = N // 128           # 12 row chunks
ACT_F = mybir.ActivationFunctionType


# ---------------------------------------------------------------- host math
def _positions_np():
    """get_positional_embed(1536, 192) from the Enformer reference, in numpy."""
    seq_len, feature_size = N, NRPF
    distances = np.arange(-seq_len + 1, seq_len, dtype=np.float64)
    absd = np.abs(distances)[:, None]
    nb = feature_size // 6
    max_range = math.log(seq_len) / math.log(2.0)
    half_life = 2.0 ** np.linspace(3.0, max_range, nb)
    pe_exp = np.exp(-math.log(2.0) / half_life[None, :] * absd)
    center_widths = 2.0 ** np.arange(1, nb + 1, dtype=np.float64) - 1.0
    pe_cm = (center_widths[None, :] > absd).astype(np.float64)
    stddev = seq_len / (2.0 * nb)
    start_mean = seq_len / nb
    mean = np.linspace(start_mean, float(seq_len), nb)[None, :]
    conc = (mean / stddev) ** 2
    rate = mean / (stddev ** 2)
    with np.errstate(divide="ignore"):
        log_unnorm = (conc - 1.0) * np.where(absd > 0, np.log(np.maximum(absd, 1e-300)), -np.inf)
    log_unnorm = np.where(absd > 0, log_unnorm, np.where(conc - 1.0 > 0, -np.inf, 0.0)) - rate * absd
    lgam = np.vectorize(math.lgamma)(conc)
    log_norm = lgam - conc * np.log(rate)
    probs = np.exp(log_unnorm - log_norm) + 1e-8
    pe_g = probs / probs.max(axis=-1, keepdims=True)
    emb = np.concatenate([pe_exp, pe_cm, pe_g], axis=-1)
    full = np.concatenate([emb, np.sign(distances)[:, None] * emb], axis=-1)
    return full.astype(np.float32)  # (3071, 192)


# ---------------------------------------------------------------- device IR
def _build_nc():
    nc = bass.Bass()
    x = nc.declare_dram_parameter("x", [N, D], F16, isOutput=False)
    wq = nc.declare_dram_parameter("wq", [D, F], F16, isOutput=False)
    wk = nc.declare_dram_parameter("wk", [D, F], F16, isOutput=False)
    wv = nc.declare_dram_parameter("wv", [D, DVG], F16, isOutput=False)
    wrel = nc.declare_dram_parameter("wrel", [NRPF, F], F16, isOutput=False)
    rcb = nc.declare_dram_parameter("rcb", [128, 2], F32, isOutput=False)
    rpb = nc.declare_dram_parameter("rpb", [128, 2], F32, isOutput=False)
    wout = nc.declare_dram_parameter("wout", [DVG, D], F16, isOutput=False)
    bvec = nc.declare_dram_parameter("bvec", [D], F32, isOutput=False)
    post = nc.declare_dram_parameter("post", [NRPF, RP], F16, isOutput=False)
    out = nc.declare_dram_parameter("out", [N, D], F16, isOutput=True)

    with tile.TileContext(nc) as tc, ExitStack() as ctx:
        sing = ctx.enter_context(tc.tile_pool(name="sing", bufs=1))
        pbs = ctx.enter_context(tc.tile_pool(name="pbs", bufs=1))
        dscr = ctx.enter_context(tc.tile_pool(name="dscr", bufs=8, space="DRAM"))

        # ---- persistent constants
        ident16 = sing.tile([128, 128], F16, tag="ident")
        make_identity(nc, ident16[:, :])
        rcb32 = sing.tile([128, 2], F32, tag="rcb")
        rpb32 = sing.tile([128, 2], F32, tag="rpb")
        nc.sync.dma_start(out=rcb32, in_=rcb[:, :])
        nc.sync.dma_start(out=rpb32, in_=rpb[:, :])
        woutb = sing.tile([128, 6, D], F16, tag="wob", name="woutb")
        wout16 = [woutb[:, dvt, :] for dvt in range(6)]
        relkT = [sing.tile([128, RP], F16, tag=f"relkT{ft}", name=f"relkT_{ft}")
                 for ft in range(2)]
        qcT = [sing.tile([128, N], F16, tag=f"qcT{t}", name=f"qcT_{t}") for t in range(2)]
        qbT = [sing.tile([128, N], F16, tag=f"qbT{t}", name=f"qbT_{t}") for t in range(2)]
        kT = [sing.tile([128, N], F16, tag=f"kT{t}", name=f"kT_{t}") for t in range(2)]
        v16 = [sing.tile([128, DVG], BF16, tag=f"v{c}", name=f"v16_{c}") for c in range(NT)]
        outtT = [sing.tile([128, 512], F16, tag=f"outt{k}", name=f"outtT_{k}")
                 for k in range(6)]

        scrb = [dscr.tile([4 * 128 * BW], F16, tag=f"scr{bi}", name=f"scr_{bi}")
                for bi in range(12)]

        bs16_tiles = {}

        def bs16_prefetch(hb):
            # batched shifted-band read-back (2 row-chunks per DMA), on the
            # scalar HWDGE ring
            if hb >= 24 or hb in bs16_tiles:
                return
            t = pbs.tile([128, 2, N], F16, tag="bs16", bufs=4, name=f"bs16_{hb}")
            bs16_tiles[hb] = t
            nc.scalar.dma_start(
                out=t,
                in_=bass.AP(tensor=scrb[hb // 2][:].tensor,
                            offset=(hb % 2) * 2 * 128 * BW + 127,
                            ap=[[BW - 1, 128], [128 * BW, 2], [1, N]]))

        # ========== phase A: projections + rel_k + band prepass ============
        with tc.tile_pool(name="pa", bufs=1) as pa, \
             tc.tile_pool(name="psA", bufs=2, space="PSUM") as ps_small, \
             tc.tile_pool(name="psBd", bufs=3, space="PSUM") as ps_band:
            # highest-priority loads: rel positional tables (first matmuls)
            # on the scalar ring; x/weights on the sync ring (both HWDGE)
            wrel16a = pa.tile([128, F], F16, tag="wrela")
            wrel16b = pa.tile([64, F], F16, tag="wrelb")
            nc.scalar.dma_start(out=wrel16a, in_=wrel[0:128, :])
            nc.scalar.dma_start(out=wrel16b, in_=wrel[128:192, :])
            post16a = pa.tile([128, RP], F16, tag="posta")
            post16b = pa.tile([64, RP], F16, tag="postb")
            nc.scalar.dma_start(out=post16a, in_=post[0:128, :])
            nc.scalar.dma_start(out=post16b, in_=post[128:192, :])

            xp = {}

            def xp_load(w):
                t = pa.tile([128, 4, D], F16, tag="xp", bufs=2, name=f"xp_{w}")
                xp[w] = t
                for half in range(2):
                    nc.gpsimd.dma_start(
                        out=t[:, 2 * half:2 * half + 2, :],
                        in_=x[w * 512 + half * 256:w * 512 + (half + 1) * 256,
                              :].rearrange("(m p) d -> p m d", p=128))

            xp_load(0)
            wq16b = pa.tile([128, NT, F], F16, tag="wqb", name="wq16b")
            nc.gpsimd.dma_start(out=wq16b, in_=wq[:, :].rearrange("(t p) f -> p t f", p=128))
            wk16b = pa.tile([128, NT, F], F16, tag="wkb", name="wk16b")
            nc.gpsimd.dma_start(out=wk16b, in_=wk[:, :].rearrange("(t p) f -> p t f", p=128))
            wv16b = pa.tile([128, NT, DVG], F16, tag="wvb", name="wv16b")
            nc.gpsimd.dma_start(out=wv16b, in_=wv[:, :].rearrange("(t p) f -> p t f", p=128))
            xp_load(1)
            wq16 = [wq16b[:, dt, :] for dt in range(NT)]
            wk16 = [wk16b[:, dt, :] for dt in range(NT)]
            wv16 = [wv16b[:, dt, :] for dt in range(NT)]

            # PE warm-up: dense junk matmuls bridge the load wait and lift
            # the HAM clock-gate to 8/8 before the real work starts
            wu_anchor = pa.tile([128, 128], F16, tag="wua", name="wu_anchor")
            for wi in range(48):
                pwu = ps_small.tile([128, 512], F32, tag="small", name="pwu")
                nc.tensor.matmul(pwu[:, 0:128], ident16, ident16,
                                 start=True, stop=True)
                if wi >= 46:
                    nc.vector.tensor_copy(wu_anchor, pwu[:, 0:128])

            # rel_k first: a dense matmul warm-up block with the cheapest deps
            for ft in range(2):
                for rw in range(6):
                    pr = ps_small.tile([128, 512], F32, tag="small", name="pr")
                    nc.tensor.matmul(pr, wrel16a[:, ft * 128:(ft + 1) * 128],
                                     post16a[:, rw * 512:(rw + 1) * 512],
                                     start=True, stop=False)
                    nc.tensor.matmul(pr, wrel16b[:, ft * 128:(ft + 1) * 128],
                                     post16b[:, rw * 512:(rw + 1) * 512],
                                     start=False, stop=True)
                    if (ft * 6 + rw) % 2 == 0:
                        nc.vector.tensor_copy(relkT[ft][:, rw * 512:(rw + 1) * 512], pr)
                    else:
                        nc.scalar.copy(relkT[ft][:, rw * 512:(rw + 1) * 512], pr)

            def emit_window(w):
                if w < 2:  # prefetch next window's x load
                    xp_load(w + 1)
                xts = pa.tile([128, 4, NT, 128], F16, tag="xts", bufs=2,
                              name=f"xts_{w}")
                for cc in range(2):
                    nc.sync.dma_start_transpose(
                        xts[:, 2 * cc:2 * cc + 2, :, :],
                        xp[w][:, 2 * cc:2 * cc + 2, :])
                for ft in range(2):
                    pq = ps_small.tile([128, 512], F32, tag="small", name="pq")
                    for dt in range(NT):
                        nc.tensor.matmul(pq, wq16[dt][:, ft * 128:(ft + 1) * 128],
                                         xts[:, :, dt, :],
                                         start=(dt == 0), stop=(dt == NT - 1))
                    nc.vector.tensor_scalar(out=qcT[ft][:, w * 512:(w + 1) * 512], in0=pq,
                                            scalar1=0.125, scalar2=rcb32[:, ft:ft + 1],
                                            op0=mybir.AluOpType.mult, op1=mybir.AluOpType.add)
                    nc.vector.tensor_scalar(out=qbT[ft][:, w * 512:(w + 1) * 512], in0=pq,
                                            scalar1=0.125, scalar2=rpb32[:, ft:ft + 1],
                                            op0=mybir.AluOpType.mult, op1=mybir.AluOpType.add)
                    pk = ps_small.tile([128, 512], F32, tag="small", name="pk")
                    for dt in range(NT):
                        nc.tensor.matmul(pk, wk16[dt][:, ft * 128:(ft + 1) * 128],
                                         xts[:, :, dt, :],
                                         start=(dt == 0), stop=(dt == NT - 1))
                    nc.scalar.copy(kT[ft][:, w * 512:(w + 1) * 512], pk)
                for c in range(4):
                    pva = ps_small.tile([128, 512], F32, tag="small", name="pva")
                    pvb = ps_small.tile([128, 512], F32, tag="small", name="pvb")
                    for dt in range(NT):
                        nc.tensor.matmul(pva, xts[:, c, dt, :],
                                         wv16[dt][:, 0:512], start=(dt == 0), stop=(dt == NT - 1))
                        nc.tensor.matmul(pvb[:, 0:256], xts[:, c, dt, :],
                                         wv16[dt][:, 512:768], start=(dt == 0), stop=(dt == NT - 1))
                    nc.vector.tensor_copy(v16[w * 4 + c][:, 0:512], pva)
                    nc.scalar.copy(v16[w * 4 + c][:, 512:768], pvb[:, 0:256])

            def band_batch(s, hh):
                # 4 row-chunks (c = 0..3) of one (section, head-pair-half):
                # matmuls + evacs per chunk, DRAM write per chunk-pair
                iw, p = divmod(s, 2)
                h = 2 * p + hh
                ft, base = h // 2, (h % 2) * 64
                bi = s * 2 + hh
                evac = [nc.vector.tensor_copy, nc.scalar.copy]
                band16 = None
                for c in range(4):
                    if c % 2 == 0:
                        band16 = pa.tile([128, 2, BW], F16, tag="band16", bufs=2,
                                         name=f"band16_{bi}_{c}")
                    I = iw * 4 + c
                    r0 = 1408 - 128 * I
                    lhs = qbT[ft][base:base + 64, I * 128:(I + 1) * 128]
                    ps0 = ps_band.tile([128, 1024], F32, tag="bh", name="ps0")
                    nc.tensor.matmul(ps0[:, 0:512], lhs,
                                     relkT[ft][base:base + 64, r0:r0 + 512],
                                     start=True, stop=True)
                    nc.tensor.matmul(ps0[:, 512:1024], lhs,
                                     relkT[ft][base:base + 64, r0 + 512:r0 + 1024],
                                     start=True, stop=True)
                    evac[c % 2](band16[:, c % 2, 0:1024], ps0)
                    ps1 = ps_band.tile([128, 1024], F32, tag="bh", name="ps1")
                    nc.tensor.matmul(ps1[:, 0:512], lhs,
                                     relkT[ft][base:base + 64, r0 + 1024:r0 + 1536],
                                     start=True, stop=True)
                    nc.tensor.matmul(ps1[:, 512:640], lhs,
                                     relkT[ft][base:base + 64, r0 + 1536:r0 + 1664],
                                     start=True, stop=True)
                    evac[(c + 1) % 2](band16[:, c % 2, 1024:1664], ps1[:, 0:640])
                    if c % 2 == 1:
                        nc.sync.dma_start(
                            out=bass.AP(tensor=scrb[bi][:].tensor,
                                        offset=(c - 1) * 128 * BW,
                                        ap=[[BW, 128], [128 * BW, 2], [1, BW]]),
                            in_=band16)

            emit_window(0)
            for s_ in (0, 1):
                for hh in range(2):
                    band_batch(s_, hh)
                if s_ == 0:
                    for hb0 in range(4):
                        bs16_prefetch(hb0)
            emit_window(1)
            for s_ in (2, 3):
                for hh in range(2):
                    band_batch(s_, hh)
            emit_window(2)
            for s_ in (4, 5):
                for hh in range(2):
                    band_batch(s_, hh)

        # ================= phases B/C/D: attention (scoped SBUF) ===========
        with tc.tile_pool(name="pb", bufs=1) as pb, \
             tc.tile_pool(name="psBig", bufs=2, space="PSUM") as ps_big, \
             tc.tile_pool(name="psSm", bufs=2, space="PSUM") as ps_small:
            nc.gpsimd.dma_start(out=woutb,
                                in_=wout[:, :].rearrange("(t p) d -> p t d", p=128))
            at2 = [[pb.tile([128, 4, NT, 128], BF16, tag=f"ats{g}{hh}", name=f"ats_{g}_{hh}")
                    for hh in range(2)] for g in range(2)]

            def soft_stage(s, hh, c):
                iw, p = divmod(s, 2)
                g = s % 2
                h = 2 * p + hh
                ft, base = h // 2, (h % 2) * 64
                I = iw * 4 + c
                it = s * 8 + hh * 4 + c
                hb = it // 2
                bs16 = bs16_tiles[hb]
                psc = ps_big.tile([128, N], F32, tag="big", name="psc")
                for jw in range(3):
                    nc.tensor.matmul(
                        psc[:, jw * 512:(jw + 1) * 512],
                        qcT[ft][base:base + 64, I * 128:(I + 1) * 128],
                        kT[ft][base:base + 64, jw * 512:(jw + 1) * 512],
                        start=True, stop=False)
                for jw in range(3):
                    nc.tensor.matmul(
                        psc[:, jw * 512:(jw + 1) * 512],
                        ident16,
                        bs16[:, it % 2, jw * 512:(jw + 1) * 512],
                        start=False, stop=True)
                if it % 2 == 1:
                    bs16_tiles.pop(hb)
                    bs16_prefetch(hb + 4)
                a16 = pb.tile([128, N], BF16, tag="a16", bufs=4, name="a16")
                s32 = pb.tile([128, 1], F32, tag="s32", bufs=3, name="s32")
                nc.scalar.activation(out=a16, in_=psc, func=ACT_F.Exp,
                                     accum_out=s32)
                rs32 = pb.tile([128, 1], F32, tag="rs32", bufs=3, name="rs32")
                nc.vector.reciprocal(rs32, s32)
                a16n = pb.tile([128, N], BF16, tag="a16n", bufs=4, name="a16n")
                nc.vector.tensor_scalar_mul(a16n, a16, rs32)
                nc.sync.dma_start_transpose(at2[g][hh][:, c, :, :], a16n)

            def pv_group(s, k):
                # one dv-column k-group of attn@v for section s (12 matmuls)
                iw, p = divmod(s, 2)
                g = s % 2
                h0c, h1c = (2 * p) * DV, (2 * p + 1) * DV
                pspv = ps_small.tile([128, 512], F32, tag="small", name="pspv")
                for jt in range(NT):
                    st, sp = jt == 0, jt == NT - 1
                    at0 = at2[g][0][:, :, jt, :]
                    at1 = at2[g][1][:, :, jt, :]
                    if k == 0:
                        nc.tensor.matmul(pspv, v16[jt][:, h0c:h0c + 128],
                                         at0, start=st, stop=sp)
                    elif k == 2:
                        nc.tensor.matmul(pspv, v16[jt][:, h1c + 64:h1c + 192],
                                         at1, start=st, stop=sp)
                    else:
                        nc.tensor.matmul(pspv[0:64, :], v16[jt][:, h0c + 128:h0c + 192],
                                         at0, start=st, stop=sp, tile_position=(0, 0))
                        nc.tensor.matmul(pspv[64:128, :], v16[jt][:, h1c:h1c + 64],
                                         at1, start=st, stop=sp, tile_position=(0, 64))
                if k % 2 == 0:
                    nc.vector.tensor_copy(outtT[p * 3 + k], pspv)
                else:
                    nc.scalar.copy(outtT[p * 3 + k], pspv)

            of_tiles = {}

            def out_col(iw, c):
                # one 128-row output chunk of the final projection (18 matmuls)
                if c == 0:
                    of_tiles[iw] = pb.tile([128, 4, N], F16, tag="of", bufs=1,
                                           name=f"of_{iw}")
                of4 = of_tiles[iw]
                for jw in range(3):
                    pso = ps_small.tile([128, 512], F32, tag="small", name="pso")
                    for dvt in range(6):
                        nc.tensor.matmul(pso,
                                         outtT[dvt][:, c * 128:(c + 1) * 128],
                                         wout16[dvt][:, jw * 512:(jw + 1) * 512],
                                         start=(dvt == 0), stop=(dvt == 5))
                    if jw % 2 == 0:
                        nc.vector.tensor_copy(of4[:, c, jw * 512:(jw + 1) * 512], pso)
                    else:
                        nc.scalar.copy(of4[:, c, jw * 512:(jw + 1) * 512], pso)
                if c == 3:
                    nc.gpsimd.dma_start(
                        out=bass.AP(tensor=out[:, :].tensor, offset=iw * 512 * D,
                                    ap=[[D, 128], [128 * D, 4], [1, D]]),
                        in_=of4)

            # PE filler work, in dependency order. pv_group(s, k) is legal
            # once section s's transposes are done (one section later);
            # out_col(iw, c) needs pv 2*iw and 2*iw+1 evacuated.
            fillers = {
                1: [lambda k=k: pv_group(0, k) for k in range(3)],
                2: [lambda k=k: pv_group(1, k) for k in range(3)],
                3: [lambda c=c: out_col(0, c) for c in range(4)]
                   + [lambda k=k: pv_group(2, k) for k in range(3)],
                4: [lambda k=k: pv_group(3, k) for k in range(3)]
                   + [lambda c=c: out_col(1, c) for c in range(4)],
                5: [lambda k=k: pv_group(4, k) for k in range(3)],
            }

            def emit_B(s):
                fl = list(fillers.get(s, []))
                # spread fillers across the section's 8 stages, back-loaded
                # so their inputs (previous section's evacs) are ready
                slots = {7 - i: fl[len(fl) - 1 - i] for i in range(len(fl))}
                j = 0
                for hh in range(2):
                    for c in range(4):
                        soft_stage(s, hh, c)
                        if j in slots:
                            slots[j]()
                        j += 1

            for s_ in range(6):
                emit_B(s_)
            for k in range(3):
                pv_group(5, k)
            for c in range(4):
                out_col(2, c)
    return nc


# --------------------------------------------------- multi-wait legalization
_legal_counter = [0]


def _legalize_multi_waits(nc, max_waits=1, max_updates=1):
    """Split multi-wait/update instructions into EventSemaphore chains.

    The TRN2 instruction encoding holds one sync-wait and one sync-update
    command; Tile attaches as many as the dependence structure needs, so we
    hoist the extras onto standalone EventSemaphore instructions."""
    for f in nc.m.functions:
        for blk in f.blocks:
            outl, changed = [], False
            for inst in blk.instructions:
                si = inst.sync_info
                if si is not None and si.on_wait and len(si.on_wait) > max_waits:
                    waits = list(si.on_wait)
                    for wcmd in waits[:-max_waits]:
                        ev = mybir.InstEventSemaphore(
                            name=f"legalw-{_legal_counter[0]}", ins=[], outs=[])
                        _legal_counter[0] += 1
                        ev.engine = inst.engine
                        ev.sync_info = mybir.SyncInfo(on_wait=[wcmd], on_update=[])
                        outl.append(ev)
                        changed = True
                    inst.sync_info = mybir.SyncInfo(
                        on_wait=waits[-max_waits:], on_update=list(si.on_update or []))
                    si = inst.sync_info
                if si is not None and si.on_update and len(si.on_update) > max_updates:
                    ups = list(si.on_update)
                    inst.sync_info = mybir.SyncInfo(
                        on_wait=list(si.on_wait or []), on_update=ups[:max_updates])
                    outl.append(inst)
                    for ucmd in ups[max_updates:]:
                        ev = mybir.InstEventSemaphore(
                            name=f"legalu-{_legal_counter[0]}", ins=[], outs=[])
                        _legal_counter[0] += 1
                        ev.engine = inst.engine
                        ev.sync_info = mybir.SyncInfo(on_wait=[], on_update=[ucmd])
                        outl.append(ev)
                    changed = True
                    continue
                outl.append(inst)
            if changed:
                blk.instructions = outl
    return nc


# ------------------------------------------------------------------- driver
_NC_CACHE = {}
LAST = {}


def _get_nc():
    if "nc" not in _NC_CACHE:
        nc = _build_nc()
        _legalize_multi_waits(nc)
        _NC_CACHE["nc"] = nc
    return _NC_CACHE["nc"]


def kernel(x, Wq, Wk, Wv, Wrel, rel_content_bias, rel_pos_bias, Wout, b_out):
    x16 = np.ascontiguousarray(np.asarray(x, dtype=np.float32)).astype(np.float16)
    Wq = np.asarray(Wq, dtype=np.float32).astype(np.float16)
    Wk = np.asarray(Wk, dtype=np.float32).astype(np.float16)
    Wv = np.asarray(Wv, dtype=np.float32).astype(np.float16)
    Wrel = np.asarray(Wrel, dtype=np.float32).astype(np.float16)
    rcb = np.asarray(rel_content_bias, dtype=np.float32).reshape(H, DK)
    rpb = np.asarray(rel_pos_bias, dtype=np.float32).reshape(H, DK)
    Wout = np.asarray(Wout, dtype=np.float32).astype(np.float16)
    b_out = np.asarray(b_out, dtype=np.float32)

    post = np.zeros((NRPF, RP), dtype=np.float32)
    post[:, :R] = _positions_np().T
    post = post.astype(np.float16)

    zeros = np.zeros_like(b_out)
    in_maps = []
    for core in range(8):
        b, g = core // 2, core % 2
        f0, v0 = g * F, g * DVG
        rcb_g = np.zeros((128, 2), dtype=np.float32)
        rpb_g = np.zeros((128, 2), dtype=np.float32)
        for t in range(2):
            rcb_g[:, t] = rcb[g * HG + 2 * t: g * HG + 2 * t + 2].reshape(128)
            rpb_g[:, t] = rpb[g * HG + 2 * t: g * HG + 2 * t + 2].reshape(128)
        in_maps.append({
            "x": np.ascontiguousarray(x16[b]),
            "wq": np.ascontiguousarray(Wq[:, f0:f0 + F]),
            "wk": np.ascontiguousarray(Wk[:, f0:f0 + F]),
            "wv": np.ascontiguousarray(Wv[:, v0:v0 + DVG]),
            "wrel": np.ascontiguousarray(Wrel[:, f0:f0 + F]),
            "rcb": rcb_g,
            "rpb": rpb_g,
            "wout": np.ascontiguousarray(Wout[v0:v0 + DVG, :]),
            "bvec": b_out if g == 0 else zeros,
            "post": post,
        })

    nc = _get_nc()
    res = None
    for attempt in range(3):
        try:
            res = run_bass_kernel_spmd(nc, in_maps, list(range(8)))
            break
        except Exception:
            if attempt == 2:
                raise
    LAST["res"] = res
    parts = [res.results[c]["out"] for c in range(8)]
    out = np.empty((B, N, D), dtype=np.float32)
    for b in range(B):
        out[b] = parts[2 * b].astype(np.float32) + parts[2 * b + 1].astype(np.float32)
    out += b_out[None, None, :]
    return out
